# revision 25
# baseline (speedup 1.0000x reference)
"""Trainium2 Bass kernel for axial (per-frame) spatial multi-head attention.

Computation (per batch element b):
    qkv = x @ Wqkv ; q,k,v heads of 64 dims, q scaled by D**-0.5
    per (head, frame): attn = softmax(q @ k^T) over 196 spatial tokens
    out = attn @ v ; y = concat-heads(out) @ Wout + bout

Sharding: pure data-parallel over batch B=8 -> one NeuronCore per batch
element, no collectives. Each core computes its full [1568, 512] output.

Single-core dataflow (no on-device transposes anywhere):
  - host supplies x^T [512,1568] fp16; q/k produced TRANSPOSED (qT/kT
    [64h, t]) with Wq/k slices stationary; v produced NATURAL with xT
    slices stationary. All PE matmuls fp16/bf16 (1 cy/row).
  - P1: all 32 q/k projection chains then all 16 v chains, psum rotating
    through six slots across three pool tags so nothing waits on drains.
  - attention is FRAME-MAJOR (unit u = 4*frame + pair) and software-
    pipelined: sim(u) at step u, AV(u) at u-2 lag, normalize at u-3 lag.
    Frame-major makes the output-projection chains' dependencies finish
    progressively, so out-proj interleaves into the pipeline instead of
    bunching at the end.
  - sim^T per (pair, frame): four K=64 matmuls alternating PE quadrants
    (head-even rows 0:64, head-odd 64:128 -> loads hide under streaming);
    one ACT exp (bias=-SHIFT) over both heads writes attnwT bf16.
  - AV contracts j on partitions with a per-head ones column in v; row 64
    of the psum output is the softmax denominator.
  - normalize spreads over all four engines: one [65,392] copy of the av
    psum to SBUF (ACT/DVE alternating; also the only PSUM read - the
    custom-DVE reciprocal faults on PSUM operands), DVE reciprocal of the
    denominator row, GpSimd partition_broadcast (SBUF->SBUF), then the
    two lane-shifted muls split GpSimd (head-even, all-SBUF) / DVE.
  - TRN2 p-state needs ~3us of continuous PE busy for 2.4 GHz; the
    pipeline keeps the PE queue non-empty from the first projection to
    the last out-proj chain.
"""

import numpy as np

B, N, DIM = 8, 1568, 512
H, D, F = 8, 64, 8
NTOK = 196          # spatial tokens per frame
TCH = 392           # token chunk (2 frames), 4*392=1568
KC = 4              # 128-row chunks over DIM contraction
SHIFT = 90.0        # softmax exp shift (see module docstring)
VSTR = 65           # per-head stride in v_aug (64 dims + ones column)
NU = 4 * F          # attention units (pair, frame)

_cache = {}


def _build_bass(use_bias: bool):
    import concourse.tile as tile
    import concourse.mybir as mybir
    from concourse import bacc

    fp32 = mybir.dt.float32
    fp16 = mybir.dt.float16
    bf16 = mybir.dt.bfloat16
    Exp = mybir.ActivationFunctionType.Exp

    nc = bacc.Bacc()
    xT_d = nc.declare_dram_parameter("xT", [DIM, N], fp16, isOutput=False)
    wqkv_d = nc.declare_dram_parameter("wqkv", [DIM, 3 * DIM], fp16, isOutput=False)
    wout_d = nc.declare_dram_parameter("wout", [DIM, DIM], fp16, isOutput=False)
    if use_bias:
        bout_d = nc.declare_dram_parameter("boutr", [1, DIM], fp16, isOutput=False)
    out_d = nc.declare_dram_parameter("out", [N, DIM], fp16, isOutput=True)

    with tile.TileContext(nc) as tc:
        with (
            tc.tile_pool(name="weights", bufs=1) as wpool,
            tc.tile_pool(name="acts", bufs=1) as apool,
            tc.tile_pool(name="attnw", bufs=4) as atpool,
            tc.tile_pool(name="rows", bufs=2) as rpool,
            tc.tile_pool(name="avs", bufs=3) as avspool,
            tc.tile_pool(name="ys", bufs=4) as yspool,
            tc.tile_pool(name="pmm", bufs=2, space="PSUM") as pmm,
            tc.tile_pool(name="psim", bufs=2, space="PSUM") as psim,
            tc.tile_pool(name="pav", bufs=2, space="PSUM") as pav,
        ):
            # ---- resident loads: weights on the ACT queue, x^T on SP.
            # (GpSimd-issued DMA silently corrupts on this runtime - only
            # SP and ACT host DMAs.) q/k weight halves first: the first 32
            # chains need only wqk + the matching x^T halves. ----
            # host packs wqkv columns pair-major: [q_p0|k_p0|q_p1|k_p1|...|v]
            # so the chains' m-order consumes contiguous, early-arriving cols.
            wqk, wv = [], []
            for kc in range(KC):
                t = wpool.tile([128, 2 * DIM], fp16, tag=f"wqk_{kc}",
                               name=f"wqk_{kc}")
                if kc == 0:
                    nc.scalar.dma_start(
                        out=t[:, 0:256], in_=wqkv_d[0:128, 0:256]
                    )
                    nc.scalar.dma_start(
                        out=t[:, 256:2 * DIM], in_=wqkv_d[0:128, 256:2 * DIM]
                    )
                else:
                    nc.scalar.dma_start(
                        out=t[:], in_=wqkv_d[kc * 128:(kc + 1) * 128, 0:2 * DIM]
                    )
                wqk.append(t)
            for kc in range(KC):
                t = wpool.tile([128, DIM], fp16, tag=f"wv_{kc}", name=f"wv_{kc}")
                nc.scalar.dma_start(
                    out=t[:], in_=wqkv_d[kc * 128:(kc + 1) * 128, 2 * DIM:3 * DIM]
                )
                wv.append(t)
            xt = [wpool.tile([128, N], fp16, tag=f"xt_{kc}", name=f"xt_{kc}")
                  for kc in range(KC)]
            for nch in range(4):
                for kc in range(KC):
                    nc.sync.dma_start(
                        out=xt[kc][:, nch * TCH:(nch + 1) * TCH],
                        in_=xT_d[kc * 128:(kc + 1) * 128,
                                 nch * TCH:(nch + 1) * TCH],
                    )
            wout = []
            for p in range(4):
                t = wpool.tile([128, DIM], fp16, tag=f"wout_{p}", name=f"wout_{p}")
                nc.scalar.dma_start(out=t[:], in_=wout_d[p * 128:(p + 1) * 128, :])
                wout.append(t)
            if use_bias:
                boutt = wpool.tile([1, DIM], fp16, tag="boutr", name="boutr")
                nc.sync.dma_start(out=boutt[:], in_=bout_d[:])
                ones_r = wpool.tile([1, 128], fp16, tag="ones_r", name="ones_r")
                nc.gpsimd.memset(ones_r[:], 1.0)
            negshift = wpool.tile([128, 1], fp32, tag="negshift", name="negshift")
            nc.gpsimd.memset(negshift[:], -SHIFT)

            # qT tiles m=0..3 (pair m heads 2m,2m+1); kT tiles m=4..7 with 64
            # zero pad columns so the jc1 stationary slice of the last frame
            # stays in bounds (rows 68:128 of jc1 sim output are garbage,
            # never read downstream).
            qkvT = [apool.tile([128, N if m < 4 else N + 64], fp16,
                               tag=f"qkvT_{m}", name=f"qkvT_{m}")
                    for m in range(8)]
            for m in range(4, 8):
                nc.gpsimd.memset(qkvT[m][:, N:N + 64], 0.0)
            vaug = []
            for fr in range(F):
                pair = []
                for c, rows in ((0, 128), (1, 68)):
                    t = apool.tile([rows, H * VSTR], bf16, tag=f"vaug_{fr}_{c}",
                                   name=f"vaug_{fr}_{c}")
                    nc.gpsimd.memset(
                        t[:].rearrange("p (h c) -> p h c", h=H)[:, :, 64:65], 1.0
                    )
                    pair.append(t)
                vaug.append(pair)
            outT = [apool.tile([128, N], fp16, tag=f"outT_{p}", name=f"outT_{p}")
                    for p in range(4)]

            # psum chains rotate through six slots across the three pools so
            # P1 never waits on a drain; attention reuses sim/av tags.
            _ck = [0]
            _pools = [(pmm, "mm"), (psim, "sim"), (pav, "av")]

            def chain_ps():
                _ck[0] += 1
                return pmm.tile([128, DIM], fp32, tag="mm", name="chps")

            drains = [nc.scalar.copy, nc.vector.tensor_copy]

            def qk_chain(m, nch, drain):
                col0 = 256 * m if m < 4 else 256 * (m - 4) + 128
                ps = chain_ps()
                for kc in range(KC):
                    nc.tensor.matmul(
                        ps[:, 0:TCH],
                        wqk[kc][:, col0:col0 + 128],
                        xt[kc][:, nch * TCH:(nch + 1) * TCH],
                        start=(kc == 0), stop=(kc == KC - 1),
                    )
                drain(qkvT[m][:, nch * TCH:(nch + 1) * TCH], ps[:, 0:TCH])

            def v_chain(fr, c, drain):
                rows = 128 if c == 0 else 68
                tok0 = fr * NTOK + c * 128
                ps = chain_ps()
                for kc in range(KC):
                    nc.tensor.matmul(
                        ps[0:rows, :],
                        xt[kc][:, tok0:tok0 + rows],
                        wv[kc][:],
                        start=(kc == 0), stop=(kc == KC - 1),
                    )
                drain(
                    vaug[fr][c][:].rearrange("p (h c) -> p h c", h=H)[:, :, 0:64],
                    ps[0:rows, :].rearrange("p (h c) -> p h c", h=H),
                )

            # ---- P1: all q/k chains (nch-wave order matches x^T arrival),
            # then all v chains ----
            di = 0
            for nch in (0, 1):
                for m in (0, 4, 1, 5, 2, 6, 3, 7):
                    qk_chain(m, nch, drains[di % 2])
                    di += 1
            for fr in (0, 1):
                for c in (0, 1):
                    v_chain(fr, c, drains[di % 2])
                    di += 1

            # ---- attention pipeline, frame-major: unit u = 4*frame + pair ----
            at_t, av_t, avs_t, rr_t, rbb_t = {}, {}, {}, {}, {}

            def sim_unit(u):
                fr, p = divmod(u, 4)
                c0 = fr * NTOK
                ps = psim.tile([128, 1024], fp32, tag="sim", name="sim")
                qTt, kTt = qkvT[p], qkvT[4 + p]
                for hh, jc in ((0, 0), (1, 0), (0, 1), (1, 1)):
                    base = hh * 64
                    off = hh * 512 + jc * NTOK
                    nc.tensor.matmul(
                        ps[0:128, off:off + NTOK],
                        kTt[base:base + 64, c0 + jc * 128:c0 + jc * 128 + 128],
                        qTt[base:base + 64, c0:c0 + NTOK],
                    )
                at = atpool.tile([128, 2 * TCH], bf16, tag="at", name="at")
                nc.scalar.activation(
                    at[:].rearrange("p (b c) -> p b c", b=2),
                    ps[:].rearrange("p (b c) -> p b c", b=2)[:, :, 0:TCH],
                    Exp,
                    bias=negshift[:],
                )
                at_t[u] = at

            def av_unit(u):
                fr, p = divmod(u, 4)
                at = at_t.pop(u)
                av = pav.tile([128, TCH], fp32, tag="av", name="av")
                for hh in range(2):
                    ato = hh * TCH
                    avo = hh * NTOK
                    for c, rows in ((0, 128), (1, 68)):
                        va = vaug[fr][c][:].rearrange(
                            "p (h c) -> p h c", h=H)[:, 2 * p + hh, :]
                        nc.tensor.matmul(
                            av[0:VSTR, avo:avo + NTOK],
                            va,
                            at[0:rows, ato + c * NTOK:ato + (c + 1) * NTOK],
                            start=(c == 0), stop=(c == 1),
                        )
                # the custom-DVE reciprocal requires a base-0 SBUF operand:
                # bounce the denominator row through dsb (ACT/DVE alternate)
                dsb = rpool.tile([1, TCH], fp32, tag="dsb", name="dsb")
                if u % 2 == 0 or u >= NU - 4:
                    nc.scalar.copy(dsb[:], av[64:65, :])
                else:
                    nc.vector.tensor_copy(dsb[:], av[64:65, :])
                rr = rpool.tile([1, TCH], fp32, tag="rr", name="rr")
                nc.vector.reciprocal_approx_fast(rr[:], dsb[:])
                avs_t[u] = av
                rr_t[u] = rr

            def norm_unit(u):
                fr, p = divmod(u, 4)
                c0 = fr * NTOK
                avs = avs_t.pop(u)
                rr = rr_t.pop(u)
                rbb = rpool.tile([64, TCH], fp32, tag="rbb", name="rbb")
                # GpSimd runs ONLY partition_broadcast: mixing it with other
                # Pool ops forces a ~6us engine mode reconfig per switch.
                nc.gpsimd.partition_broadcast(rbb[:], rr[:])
                nc.vector.tensor_mul(
                    outT[p][0:64, c0:c0 + NTOK],
                    avs[0:64, 0:NTOK],
                    rbb[:, 0:NTOK],
                )
                nc.vector.tensor_mul(
                    outT[p][64:128, c0:c0 + NTOK],
                    avs[0:64, NTOK:2 * NTOK],
                    rbb[:, NTOK:2 * NTOK],
                )
                av_t.pop(u, None)

            def out_chain(mt):
                t0 = mt * 128
                msz = min(128, N - t0)
                ps = pmm.tile([128, DIM], fp32, tag="mm", name="mm")
                for p in range(4):
                    nc.tensor.matmul(
                        ps[0:msz, :],
                        outT[p][:, t0:t0 + msz],
                        wout[p][:],
                        start=(p == 0), stop=(p == 3 and not use_bias),
                    )
                if use_bias:
                    nc.tensor.matmul(
                        ps[0:msz, :], ones_r[:, 0:msz], boutt[:],
                        start=False, stop=True,
                    )
                ys = yspool.tile([128, DIM], fp16, tag="ys", name="ys")
                nc.scalar.copy(ys[0:msz, :], ps[0:msz, :])
                nc.sync.dma_start(out=out_d[t0:t0 + msz, :], in_=ys[0:msz, :])

            # out-proj chain mt needs normalized frames <= g*(mt) of every
            # pair; with frame-major units that is norm_unit(4*g*+3) at step
            # 4*g*+6, so schedule at 4*g*+7.
            out_sched = {}
            for mt in range(13):
                gstar = (128 * mt + min(128, N - 128 * mt) - 1) // NTOK
                slot = 4 * gstar + (6 if gstar >= 6 else 8)
                out_sched.setdefault(slot, []).append(mt)

            qk_order = (0, 4, 1, 5, 2, 6, 3, 7)
            vslots = (2, 3, 6, 7, 10, 11, 16, 17, 20, 21, 24, 25)
            vsched = {}
            for i, fr in enumerate(range(2, F)):
                vsched[vslots[2 * i]] = (fr, 0)
                vsched[vslots[2 * i + 1]] = (fr, 1)

            nsteps = max(NU + 3, max(out_sched) + 1)
            for s in range(nsteps):
                if s < NU:
                    sim_unit(s)
                if 0 <= s - 2 < NU:
                    av_unit(s - 2)
                if s < 16:
                    qk_chain(qk_order[s % 8], s // 8 + 2, drains[s % 2])
                if s in vsched:
                    v_chain(*vsched.pop(s), drain=drains[(s + 1) % 2])
                if 0 <= s - 3 < NU:
                    norm_unit(s - 3)
                for mt in out_sched.pop(s, []):
                    out_chain(mt)

    nc.compile()
    return nc


def _get_program(use_bias: bool):
    key = ("nc", use_bias)
    if key not in _cache:
        _cache[key] = _build_bass(use_bias)
    return _cache[key]


def kernel(x=None, Wqkv=None, Wout=None, bout=None, f=None, **_unused):
    x = np.asarray(x, np.float32)
    Wqkv = np.asarray(Wqkv, np.float32)
    Wout = np.asarray(Wout, np.float32)
    bout = np.asarray(bout, np.float32)
    assert x.shape == (B, N, DIM) and int(f) == F

    wq = Wqkv.copy()
    wq[:, :DIM] *= D ** -0.5                       # fold q scaling into Wq
    # interleave q/k pair-major: [q_p(128) | k_p(128)] per pair, then v
    qk = wq[:, :2 * DIM].reshape(DIM, 2, 4, 128)   # [dim, q/k, pair, 128]
    qk = qk.transpose(0, 2, 1, 3).reshape(DIM, 2 * DIM)
    wq = np.concatenate([qk, wq[:, 2 * DIM:]], axis=1)
    wq16 = wq.astype(np.float16)
    wout16 = Wout.astype(np.float16)
    use_bias = bool(np.any(bout != 0.0))

    nc = _get_program(use_bias)

    in_maps = []
    for b in range(B):
        m = {
            "xT": np.ascontiguousarray(x[b].T).astype(np.float16),
            "wqkv": wq16,
            "wout": wout16,
        }
        if use_bias:
            m["boutr"] = bout.reshape(1, DIM).astype(np.float16)
        in_maps.append(m)

    from concourse.bass_utils import run_bass_kernel_spmd

    res = run_bass_kernel_spmd(nc, in_maps, core_ids=list(range(B)))
    return np.stack(
        [np.asarray(res.results[b]["out"], np.float32) for b in range(B)], axis=0
    )


# revision 26
# speedup vs baseline: 1.1813x; 1.1813x over previous
"""Trainium2 Bass kernel for axial (per-frame) spatial multi-head attention.

Computation (per batch element b):
    qkv = x @ Wqkv ; q,k,v heads of 64 dims, q scaled by D**-0.5
    per (head, frame): attn = softmax(q @ k^T) over 196 spatial tokens
    out = attn @ v ; y = concat-heads(out) @ Wout + bout

Sharding: pure data-parallel over batch B=8 -> one NeuronCore per batch
element, no collectives. Each core computes its full [1568, 512] output.

Single-core dataflow (no on-device transposes anywhere):
  - host supplies x^T [512,1568] fp16; q/k produced TRANSPOSED (qT/kT
    [64h, t]) with Wq/k slices stationary; v produced NATURAL with xT
    slices stationary. All PE matmuls fp16/bf16 (1 cy/row).
  - P1: all 32 q/k projection chains then all 16 v chains, psum rotating
    through six slots across three pool tags so nothing waits on drains.
  - attention is FRAME-MAJOR (unit u = 4*frame + pair) and software-
    pipelined: sim(u) at step u, AV(u) at u-2 lag, normalize at u-3 lag.
    Frame-major makes the output-projection chains' dependencies finish
    progressively, so out-proj interleaves into the pipeline instead of
    bunching at the end.
  - sim^T per (pair, frame): four K=64 matmuls alternating PE quadrants
    (head-even rows 0:64, head-odd 64:128 -> loads hide under streaming);
    one ACT exp (bias=-SHIFT) over both heads writes attnwT bf16.
  - AV contracts j on partitions with a per-head ones column in v; row 64
    of the psum output is the softmax denominator.
  - normalize spreads over all four engines: one [65,392] copy of the av
    psum to SBUF (ACT/DVE alternating; also the only PSUM read - the
    custom-DVE reciprocal faults on PSUM operands), DVE reciprocal of the
    denominator row, GpSimd partition_broadcast (SBUF->SBUF), then the
    two lane-shifted muls split GpSimd (head-even, all-SBUF) / DVE.
  - TRN2 p-state needs ~3us of continuous PE busy for 2.4 GHz; the
    pipeline keeps the PE queue non-empty from the first projection to
    the last out-proj chain.
"""

import numpy as np

B, N, DIM = 8, 1568, 512
H, D, F = 8, 64, 8
NTOK = 196          # spatial tokens per frame
TCH = 392           # token chunk (2 frames), 4*392=1568
KC = 4              # 128-row chunks over DIM contraction
SHIFT = 90.0        # softmax exp shift (see module docstring)
VSTR = 65           # per-head stride in v_aug (64 dims + ones column)
NU = 4 * F          # attention units (pair, frame)

_cache = {}


def _build_bass(use_bias: bool):
    import concourse.tile as tile
    import concourse.mybir as mybir
    from concourse import bacc

    fp32 = mybir.dt.float32
    fp16 = mybir.dt.float16
    bf16 = mybir.dt.bfloat16
    Exp = mybir.ActivationFunctionType.Exp

    nc = bacc.Bacc()
    xT_d = nc.declare_dram_parameter("xT", [DIM, N], fp16, isOutput=False)
    wqkv_d = nc.declare_dram_parameter("wqkv", [DIM, 3 * DIM], fp16, isOutput=False)
    wout_d = nc.declare_dram_parameter("wout", [DIM, DIM], fp16, isOutput=False)
    if use_bias:
        bout_d = nc.declare_dram_parameter("boutr", [1, DIM], fp16, isOutput=False)
    out_d = nc.declare_dram_parameter("out", [N, DIM], fp16, isOutput=True)

    with tile.TileContext(nc) as tc:
        with (
            tc.tile_pool(name="weights", bufs=1) as wpool,
            tc.tile_pool(name="acts", bufs=1) as apool,
            tc.tile_pool(name="attnw", bufs=4) as atpool,
            tc.tile_pool(name="rows", bufs=2) as rpool,
            tc.tile_pool(name="avs", bufs=3) as avspool,
            tc.tile_pool(name="ys", bufs=4) as yspool,
            tc.tile_pool(name="pmm", bufs=2, space="PSUM") as pmm,
            tc.tile_pool(name="psim", bufs=2, space="PSUM") as psim,
            tc.tile_pool(name="pav", bufs=2, space="PSUM") as pav,
        ):
            # ---- resident loads: weights on the ACT queue, x^T on SP.
            # (GpSimd-issued DMA silently corrupts on this runtime - only
            # SP and ACT host DMAs.) q/k weight halves first: the first 32
            # chains need only wqk + the matching x^T halves. ----
            # host packs wqkv columns pair-major: [q_p0|k_p0|q_p1|k_p1|...|v]
            # so the chains' m-order consumes contiguous, early-arriving cols.
            wqk, wv = [], []
            for kc in range(KC):
                t = wpool.tile([128, 2 * DIM], fp16, tag=f"wqk_{kc}",
                               name=f"wqk_{kc}")
                nc.scalar.dma_start(
                    out=t[:], in_=wqkv_d[kc * 128:(kc + 1) * 128, 0:2 * DIM]
                )
                wqk.append(t)
            for kc in range(KC):
                t = wpool.tile([128, DIM], fp16, tag=f"wv_{kc}", name=f"wv_{kc}")
                nc.scalar.dma_start(
                    out=t[:], in_=wqkv_d[kc * 128:(kc + 1) * 128, 2 * DIM:3 * DIM]
                )
                wv.append(t)
            xt = [wpool.tile([128, N], fp16, tag=f"xt_{kc}", name=f"xt_{kc}")
                  for kc in range(KC)]
            for nch in range(4):
                for kc in range(KC):
                    nc.sync.dma_start(
                        out=xt[kc][:, nch * TCH:(nch + 1) * TCH],
                        in_=xT_d[kc * 128:(kc + 1) * 128,
                                 nch * TCH:(nch + 1) * TCH],
                    )
            wout = []
            for p in range(4):
                t = wpool.tile([128, DIM], fp16, tag=f"wout_{p}", name=f"wout_{p}")
                nc.scalar.dma_start(out=t[:], in_=wout_d[p * 128:(p + 1) * 128, :])
                wout.append(t)
            if use_bias:
                boutt = wpool.tile([1, DIM], fp16, tag="boutr", name="boutr")
                nc.sync.dma_start(out=boutt[:], in_=bout_d[:])
                ones_r = wpool.tile([1, 128], fp16, tag="ones_r", name="ones_r")
                nc.gpsimd.memset(ones_r[:], 1.0)
            negshift = wpool.tile([128, 1], fp32, tag="negshift", name="negshift")
            nc.gpsimd.memset(negshift[:], -SHIFT)

            # qT tiles m=0..3 (pair m heads 2m,2m+1); kT tiles m=4..7 with 64
            # zero pad columns so the jc1 stationary slice of the last frame
            # stays in bounds (rows 68:128 of jc1 sim output are garbage,
            # never read downstream).
            qkvT = [apool.tile([128, N if m < 4 else N + 64], fp16,
                               tag=f"qkvT_{m}", name=f"qkvT_{m}")
                    for m in range(8)]
            for m in range(4, 8):
                nc.gpsimd.memset(qkvT[m][:, N:N + 64], 0.0)
            vaug = []
            for fr in range(F):
                pair = []
                for c, rows in ((0, 128), (1, 68)):
                    t = apool.tile([rows, H * VSTR], bf16, tag=f"vaug_{fr}_{c}",
                                   name=f"vaug_{fr}_{c}")
                    nc.gpsimd.memset(
                        t[:].rearrange("p (h c) -> p h c", h=H)[:, :, 64:65], 1.0
                    )
                    pair.append(t)
                vaug.append(pair)
            outT = [apool.tile([128, N], fp16, tag=f"outT_{p}", name=f"outT_{p}")
                    for p in range(4)]

            # psum chains rotate through six slots across the three pools so
            # P1 never waits on a drain; attention reuses sim/av tags.
            _ck = [0]
            _pools = [(pmm, "mm"), (psim, "sim"), (pav, "av")]

            def chain_ps():
                _ck[0] += 1
                return pmm.tile([128, DIM], fp32, tag="mm", name="chps")

            drains = [nc.scalar.copy, nc.vector.tensor_copy]

            def qk_chain(m, nch, drain):
                col0 = 256 * m if m < 4 else 256 * (m - 4) + 128
                ps = chain_ps()
                for kc in range(KC):
                    nc.tensor.matmul(
                        ps[:, 0:TCH],
                        wqk[kc][:, col0:col0 + 128],
                        xt[kc][:, nch * TCH:(nch + 1) * TCH],
                        start=(kc == 0), stop=(kc == KC - 1),
                    )
                drain(qkvT[m][:, nch * TCH:(nch + 1) * TCH], ps[:, 0:TCH])

            def v_chain(fr, c, drain):
                rows = 128 if c == 0 else 68
                tok0 = fr * NTOK + c * 128
                ps = chain_ps()
                for kc in range(KC):
                    nc.tensor.matmul(
                        ps[0:rows, :],
                        xt[kc][:, tok0:tok0 + rows],
                        wv[kc][:],
                        start=(kc == 0), stop=(kc == KC - 1),
                    )
                drain(
                    vaug[fr][c][:].rearrange("p (h c) -> p h c", h=H)[:, :, 0:64],
                    ps[0:rows, :].rearrange("p (h c) -> p h c", h=H),
                )

            # ---- P1: all q/k chains (nch-wave order matches x^T arrival),
            # then all v chains ----
            di = 0
            for nch in (0, 1):
                for m in (0, 4, 1, 5, 2, 6, 3, 7):
                    qk_chain(m, nch, drains[di % 2])
                    di += 1
            for fr in (0, 1):
                for c in (0, 1):
                    v_chain(fr, c, drains[di % 2])
                    di += 1

            # ---- attention pipeline, frame-major: unit u = 4*frame + pair ----
            at_t, av_t, avs_t, rr_t, rbb_t = {}, {}, {}, {}, {}

            def sim_unit(u):
                fr, p = divmod(u, 4)
                c0 = fr * NTOK
                ps = psim.tile([128, 1024], fp32, tag="sim", name="sim")
                qTt, kTt = qkvT[p], qkvT[4 + p]
                for hh, jc in ((0, 0), (1, 0), (0, 1), (1, 1)):
                    base = hh * 64
                    off = hh * 512 + jc * NTOK
                    nc.tensor.matmul(
                        ps[0:128, off:off + NTOK],
                        kTt[base:base + 64, c0 + jc * 128:c0 + jc * 128 + 128],
                        qTt[base:base + 64, c0:c0 + NTOK],
                    )
                at = atpool.tile([128, 2 * TCH], bf16, tag="at", name="at")
                nc.scalar.activation(
                    at[:].rearrange("p (b c) -> p b c", b=2),
                    ps[:].rearrange("p (b c) -> p b c", b=2)[:, :, 0:TCH],
                    Exp,
                    bias=negshift[:],
                )
                at_t[u] = at

            def av_unit(u):
                fr, p = divmod(u, 4)
                at = at_t.pop(u)
                av = pav.tile([128, TCH], fp32, tag="av", name="av")
                for hh in range(2):
                    ato = hh * TCH
                    avo = hh * NTOK
                    for c, rows in ((0, 128), (1, 68)):
                        va = vaug[fr][c][:].rearrange(
                            "p (h c) -> p h c", h=H)[:, 2 * p + hh, :]
                        nc.tensor.matmul(
                            av[0:VSTR, avo:avo + NTOK],
                            va,
                            at[0:rows, ato + c * NTOK:ato + (c + 1) * NTOK],
                            start=(c == 0), stop=(c == 1),
                        )
                # the custom-DVE reciprocal requires a base-0 SBUF operand:
                # bounce the denominator row through dsb (ACT/DVE alternate)
                dsb = rpool.tile([1, TCH], fp32, tag="dsb", name="dsb")
                if u % 2 == 0 or u >= NU - 4:
                    nc.scalar.copy(dsb[:], av[64:65, :])
                else:
                    nc.vector.tensor_copy(dsb[:], av[64:65, :])
                rr = rpool.tile([1, TCH], fp32, tag="rr", name="rr")
                nc.vector.reciprocal_approx_fast(rr[:], dsb[:])
                avs_t[u] = av
                rr_t[u] = rr

            def norm_unit(u):
                fr, p = divmod(u, 4)
                c0 = fr * NTOK
                avs = avs_t.pop(u)
                rr = rr_t.pop(u)
                rbb = rpool.tile([64, TCH], fp32, tag="rbb", name="rbb")
                # GpSimd runs ONLY partition_broadcast: mixing it with other
                # Pool ops forces a ~6us engine mode reconfig per switch.
                nc.gpsimd.partition_broadcast(rbb[:], rr[:])
                nc.vector.tensor_mul(
                    outT[p][0:64, c0:c0 + NTOK],
                    avs[0:64, 0:NTOK],
                    rbb[:, 0:NTOK],
                )
                nc.vector.tensor_mul(
                    outT[p][64:128, c0:c0 + NTOK],
                    avs[0:64, NTOK:2 * NTOK],
                    rbb[:, NTOK:2 * NTOK],
                )
                av_t.pop(u, None)

            def out_chain(mt):
                t0 = mt * 128
                msz = min(128, N - t0)
                ps = pmm.tile([128, DIM], fp32, tag="mm", name="mm")
                for p in range(4):
                    nc.tensor.matmul(
                        ps[0:msz, :],
                        outT[p][:, t0:t0 + msz],
                        wout[p][:],
                        start=(p == 0), stop=(p == 3 and not use_bias),
                    )
                if use_bias:
                    nc.tensor.matmul(
                        ps[0:msz, :], ones_r[:, 0:msz], boutt[:],
                        start=False, stop=True,
                    )
                ys = yspool.tile([128, DIM], fp16, tag="ys", name="ys")
                nc.scalar.copy(ys[0:msz, :], ps[0:msz, :])
                nc.sync.dma_start(out=out_d[t0:t0 + msz, :], in_=ys[0:msz, :])

            # out-proj chain mt needs normalized frames <= g*(mt) of every
            # pair; with frame-major units that is norm_unit(4*g*+3) at step
            # 4*g*+6, so schedule at 4*g*+7.
            out_sched = {}
            for mt in range(13):
                gstar = (128 * mt + min(128, N - 128 * mt) - 1) // NTOK
                slot = 4 * gstar + (6 if gstar >= 6 else 8)
                out_sched.setdefault(slot, []).append(mt)

            qk_order = (0, 4, 1, 5, 2, 6, 3, 7)
            vslots = (2, 3, 6, 7, 10, 11, 16, 17, 20, 21, 24, 25)
            vsched = {}
            for i, fr in enumerate(range(2, F)):
                vsched[vslots[2 * i]] = (fr, 0)
                vsched[vslots[2 * i + 1]] = (fr, 1)

            nsteps = max(NU + 3, max(out_sched) + 1)
            for s in range(nsteps):
                if s < NU:
                    sim_unit(s)
                if 0 <= s - 2 < NU:
                    av_unit(s - 2)
                if s < 16:
                    qk_chain(qk_order[s % 8], s // 8 + 2, drains[s % 2])
                if s in vsched:
                    v_chain(*vsched.pop(s), drain=drains[(s + 1) % 2])
                if 0 <= s - 3 < NU:
                    norm_unit(s - 3)
                for mt in out_sched.pop(s, []):
                    out_chain(mt)

    nc.compile()
    return nc


def _get_program(use_bias: bool):
    key = ("nc", use_bias)
    if key not in _cache:
        _cache[key] = _build_bass(use_bias)
    return _cache[key]


def kernel(x=None, Wqkv=None, Wout=None, bout=None, f=None, **_unused):
    x = np.asarray(x, np.float32)
    Wqkv = np.asarray(Wqkv, np.float32)
    Wout = np.asarray(Wout, np.float32)
    bout = np.asarray(bout, np.float32)
    assert x.shape == (B, N, DIM) and int(f) == F

    wq = Wqkv.copy()
    wq[:, :DIM] *= D ** -0.5                       # fold q scaling into Wq
    # interleave q/k pair-major: [q_p(128) | k_p(128)] per pair, then v
    qk = wq[:, :2 * DIM].reshape(DIM, 2, 4, 128)   # [dim, q/k, pair, 128]
    qk = qk.transpose(0, 2, 1, 3).reshape(DIM, 2 * DIM)
    wq = np.concatenate([qk, wq[:, 2 * DIM:]], axis=1)
    wq16 = wq.astype(np.float16)
    wout16 = Wout.astype(np.float16)
    use_bias = bool(np.any(bout != 0.0))

    nc = _get_program(use_bias)

    in_maps = []
    for b in range(B):
        m = {
            "xT": np.ascontiguousarray(x[b].T).astype(np.float16),
            "wqkv": wq16,
            "wout": wout16,
        }
        if use_bias:
            m["boutr"] = bout.reshape(1, DIM).astype(np.float16)
        in_maps.append(m)

    from concourse.bass_utils import run_bass_kernel_spmd

    res = run_bass_kernel_spmd(nc, in_maps, core_ids=list(range(B)))
    return np.stack(
        [np.asarray(res.results[b]["out"], np.float32) for b in range(B)], axis=0
    )
